# revision 55
# baseline (speedup 1.0000x reference)
"""Trainium2 Bass kernel for single-head causal attention.

Problem: B=4, S=2048, E=1024, H=64 fp32.
  q = x@Wq+bq; k = x@Wk+bk; v = x@Wv+bv
  out = softmax(causal(q k^T / sqrt(H))) v

Sharding: 8 cores; core c = (batch b=c//2, query-half h=c%2).
Each core computes full K/V for its batch but attention for only its
1024 queries (h=0: front 256 of each 512-tile, h=1: back 256).
SPMD-uniform: the per-core query selection is realized by a host-side
128-column block permutation of x^T (h=1 swaps the half-tiles within
each 512 tile), and causality by per-core mask tables; the device
program is identical on all cores.

All matmul operands are bf16 (1 cycle/col on the PE vs 4 for fp32),
fp32 accumulation in PSUM. x is transposed and cast to bf16 on the
host: no PE transposes of x, and DMA traffic halves (4MB/core).
The device returns pv^T tiles [65, 256] per q-tile (row 64 = softmax
denominator from a ones-column in V); the host does the final
divide + transpose, removing the whole output transpose stage.

Engine assignment: PE = projections + scores + PV + V transposes
(plus a HAM-warmup matmul burst during the initial DMA wait);
ACT = exp (512-wide pairs) + Q copy; DVE = K/V copies, masks, V-block
copies, pv copies, bias cast. K^T/Q^T live on partitions 64:127
([Wv|Wk] packing) so every PSUM->SBUF copy is partition-aligned.
(gpsimd cannot touch PSUM on HW, so it only does memsets.)

The kernel is one software-pipelined stream over 20 score/PV "pairs"
(2 k-blocks each); rounds 2 and 3 are interleaved pair-by-pair so the
exp (ACT) load of round 3 overlaps PE-heavy projection work, and an
unmasked pair closes round 3 so the final exp->mask->PV tail is short.
Projection closures for future rounds (split into chunk-gated groups
A/K1/K2/T placed at slots matching the xt DMA arrival order) and
output stages are used as PE filler between pairs, so the in-order PE
queue never stalls on the exp->mask->PV chain (PDEPTH=2 lookahead).
PSUM banks (8): scores 2 + pkv 1 + ppv 2 + pq 1 + V-trans 2.
"""

import sys
from collections import deque
from contextlib import ExitStack

import numpy as np
import ml_dtypes

if "/opt/trn_rl_repo" not in sys.path:
    sys.path.insert(0, "/opt/trn_rl_repo")

import concourse.bacc as bacc
import concourse.mybir as mybir
import concourse.tile as tile

B, S, E, H = 4, 2048, 1024, 64
NCORES = 8
F32 = mybir.dt.float32
BF16 = mybir.dt.bfloat16
AF = mybir.ActivationFunctionType
NPBF16 = ml_dtypes.bfloat16

ST = 512          # s-tile width (projections / one round)
NST = S // ST     # 4 rounds
NEC = E // 128    # 8 e-chunks (contraction)
QW = 256          # per-core q-tile width in attention
PW = 2 * QW       # paired width (2 k-blocks per exp)
XCH = 256         # xt DMA chunk width
NCH = S // XCH    # 8 chunks
PDEPTH = 3        # attention software-pipeline depth, in pairs
NWARM = 8         # HAM-warmup matmuls (512 cols each)

# head tensor columns: [wq | biases+sel | x-chunk0] — one contiguous
# first DMA so the Q pass unblocks as early as possible. (identb is
# generated on device.)
H_Q = 0
H_BVK = H_Q + NEC * H
H_X0 = H_BVK + 3
HEADC = H_X0 + NEC * XCH
# wrest tensor: [wkv] (causal masks are generated on device:
# affine_select for the diagonal pairs, exp-bias for the off pairs)
R_KV = 0
RESTC = R_KV + NEC * 128

# rounds sequential (frees a ppv PSUM bank -> 3 score banks, so the
# exp->scores chain has a whole extra pair of slack); the final pairs
# are ordered so an unmasked pair closes round 3 (short tail chain).
PAIR_ORDER = [(0, 0), (0, 1),
              (1, 0), (1, 1), (1, 2), (1, 3),
              (2, 0), (2, 1), (2, 2), (2, 3), (2, 4), (2, 5),
              (3, 0), (3, 1), (3, 2), (3, 6), (3, 7), (3, 3), (3, 5),
              (3, 4)]
# last-emitted pair per round (closes that round's PV accumulation)
LAST_PAIR = {0: 1, 1: 3, 2: 5, 3: 4}


def build_program():
    nc = bacc.Bacc("TRN2", target_bir_lowering=False, debug=False,
                   num_devices=NCORES)

    hd_d = nc.dram_tensor("head", [128, HEADC], BF16, kind="ExternalInput")
    wr_d = nc.dram_tensor("wrest", [128, RESTC], BF16, kind="ExternalInput")
    xt_d = nc.dram_tensor("xt", [128, NCH - 1, NEC, XCH], BF16,
                          kind="ExternalInput")
    y_d = nc.dram_tensor("y", [H + 1, NST, QW], F32, kind="ExternalOutput")

    with tile.TileContext(nc) as tc, ExitStack() as ctx:
        singles = ctx.enter_context(tc.tile_pool(name="singles", bufs=1))
        vtpool = ctx.enter_context(tc.tile_pool(name="vtpool", bufs=2))
        ppool = ctx.enter_context(tc.tile_pool(name="ppool", bufs=5))
        opool = ctx.enter_context(tc.tile_pool(name="opool", bufs=4))
        # PSUM 8 banks: sc-pairs 3 + kv-halves 2 + ppv 1 + pq 1 + vtrans 1
        psA = ctx.enter_context(tc.tile_pool(name="psA", bufs=3, space="PSUM"))
        psKV = ctx.enter_context(tc.tile_pool(name="psKV", bufs=2,
                                              space="PSUM"))
        psB = ctx.enter_context(tc.tile_pool(name="psB", bufs=1, space="PSUM"))
        psQ = ctx.enter_context(tc.tile_pool(name="psQ", bufs=1, space="PSUM"))
        psC = ctx.enter_context(tc.tile_pool(name="psC", bufs=1, space="PSUM"))

        # ---- SBUF tiles ----
        hd = singles.tile([128, HEADC], BF16)
        wr = singles.tile([128, RESTC], BF16)
        xt = singles.tile([128, NCH - 1, NEC, XCH], BF16)

        # ---- DMAs, in dependency-priority order (one sync ring =
        # strict FIFO = bandwidth priority). All runs are contiguous
        # multi-KiB per partition.
        def xt_dma(xc):
            nc.sync.dma_start(out=xt[:, xc - 1], in_=xt_d[:, xc - 1])

        # head in two pieces: the Q pass can start on [wq|x0 ec0-3]
        # while [x0 ec4-7] still streams.
        H1C = H_X0 + (NEC // 2) * XCH
        nc.sync.dma_start(out=hd[:, 0:H1C], in_=hd_d[:, 0:H1C])
        nc.sync.dma_start(out=hd[:, H1C:HEADC], in_=hd_d[:, H1C:HEADC])
        nc.sync.dma_start(out=wr[:, R_KV:RESTC], in_=wr_d[:, R_KV:RESTC])
        # x1 in ec-halves: K(0,1)'s first matmuls unblock half a
        # transfer earlier.
        nc.sync.dma_start(out=xt[:, 0, 0:4], in_=xt_d[:, 0, 0:4])
        nc.sync.dma_start(out=xt[:, 0, 4:8], in_=xt_d[:, 0, 4:8])
        xt_dma(2)
        xt_dma(3)
        xt_dma(4)
        xt_dma(5)
        xt_dma(6)
        xt_dma(7)

        def wkv_ap(ec):   # [Wv | Wk] chunk: out rows 0:64 = V, 64:128 = K
            return wr[:, R_KV + ec * 128: R_KV + (ec + 1) * 128]

        def wq_ap(ec):
            return hd[:, H_Q + ec * H: H_Q + (ec + 1) * H]

        def xt_ap(ch, ec):  # [128, 256] moving slab of x^T
            if ch == 0:
                return hd[:, H_X0 + ec * XCH: H_X0 + (ec + 1) * XCH]
            return xt[:, ch - 1, ec, :]

        # identity (for PE transposes) built on device on partitions
        # 64:128 (same base as the vt stationary): ones, then keep only
        # the diagonal (iota p - c == 0; partition index is view-rel).
        identb = singles.tile([128, H], BF16)
        nc.gpsimd.memset(identb[64:128, :], 1.0)
        nc.gpsimd.affine_select(identb[64:128, :], identb[64:128, :],
                                [[-1, H]], mybir.AluOpType.is_equal,
                                0.0, base=0, channel_multiplier=1)
        # HAM warmup: dummy matmuls on a just-memset tile run while the
        # first DMAs stream, so the PE clock-gate is already released
        # (and the sim's p-state ramp elapsed) when real work arrives.
        # 512-col matmuls keep the PE busy window unbroken until the
        # first xt chunk lands (~2.5 us after the PE clears its
        # preamble).
        warm = singles.tile([128, 512], BF16)
        nc.vector.memset(warm, 0.0)
        for w in range(NWARM):
            pw_ = psA.tile([128, PW], F32, tag="big", name=f"warm_{w}")
            nc.tensor.matmul(pw_, warm[:, 0:128], warm,
                             start=True, stop=True)
        # biases ride in the head as bf16; engines need fp32 scalars
        fb = singles.tile([128, 3], F32)
        nc.vector.tensor_copy(fb, hd[:, H_BVK:H_BVK + 3])
        bvk = fb[:, 0:1]        # rows 0:64 = bv, rows 64:128 = bk
        bq = fb[0:H, 1:2]
        # per-core selector bias for off-diagonal pairs: 0 keeps the
        # pair (h=1: all-causal), -10000 zeroes it via exp underflow
        # (h=0: fully masked).
        selb = fb[:, 2:3]

        qt_all = singles.tile([64, NST, QW], BF16)   # own Q^T (parts 0:64)
        kt_all = singles.tile([64, S], BF16)         # K^T (parts 0:64)
        v_all = singles.tile([128, S // 128, H + 1], BF16)  # V blocks
        nc.vector.memset(v_all[:, :, H:H + 1], 1.0)

        # ---- emission helpers ----
        # Projection work for round t is split into chunk-gated groups so
        # the in-order PE queue never blocks on a not-yet-arrived xt chunk:
        #   A(t)  = Q pass + qt copy            (needs xt chunk 2t)
        #   K1(t) = KV pass cols 0:256 + copies (needs xt chunk 2t)
        #   K2(t) = KV pass cols 256:512        (needs xt chunk 2t+1)
        #   T1/T2(t) = V-block transposes for each half
        proj_states = {}

        def emit_A(t):
            def f():
                st_ = proj_states.setdefault(t, {})
                st_["pq"] = psQ.tile([H, QW], F32, tag="pq",
                                     name=f"pq_{t}")
                for ec in range(NEC):
                    nc.tensor.matmul(st_["pq"], wq_ap(ec),
                                     xt_ap(2 * t, ec),
                                     start=(ec == 0), stop=(ec == NEC - 1))
                # partition-aligned DVE copy (0:64 -> 0:64); keeps the
                # qt chain off the ACT queue where exps would delay it.
                nc.vector.tensor_scalar_add(qt_all[:, t, :], st_["pq"], bq)
            return [f]

        def emit_K(t, s):
            c0, c1 = s * QW, (s + 1) * QW
            def f():
                st_ = proj_states.setdefault(t, {})
                if s == 0:
                    st_["vt"] = vtpool.tile([128, ST], BF16, tag="vt",
                                            name=f"vt_{t}")
                # per-half PSUM tile: its bank is free of the other
                # half's DVE reads, so K2 matmuls never serialize
                # against K1's kt/vt copies.
                pkv = psKV.tile([128, QW], F32, tag="kv",
                                name=f"pkv_{t}_{s}")
                for ec in range(NEC):
                    nc.tensor.matmul(pkv, wkv_ap(ec),
                                     xt_ap(2 * t + s, ec),
                                     start=(ec == 0), stop=(ec == NEC - 1))
                # kt first: scores depend on it; vt only feeds the
                # later V-transpose stage. [Wk|Wv] packing: K = rows
                # 0:64 -> kt_all parts 0:64, V = rows 64:128 -> vt
                # parts 64:128; both copies partition-aligned.
                nc.vector.tensor_scalar_add(
                    kt_all[:, t * ST + c0:t * ST + c1],
                    pkv[0:H, :], bvk[0:H, :])
                nc.vector.tensor_scalar_add(st_["vt"][64:128, c0:c1],
                                            pkv[64:128, :],
                                            bvk[64:128, :])
            return [f]

        def emit_K3():
            # round 3's KV pass arrives last (chunks 6,7 both present
            # by then): one 512-col-moving matmul chain halves the
            # instruction count and hides every LDWEIGHTS.
            def f():
                st_ = proj_states.setdefault(3, {})
                st_["vt"] = vtpool.tile([128, ST], BF16, tag="vt",
                                        name="vt_3")
                pkv = psKV.tile([128, ST], F32, tag="kv", name="pkv_3m")
                for ec in range(NEC):
                    nc.tensor.matmul(pkv, wkv_ap(ec), xt[:, 5:7, ec, :],
                                     start=(ec == 0), stop=(ec == NEC - 1))
                # kt halves separately: scores (3,6) needs only blocks
                # 12,13 and shouldn't wait a full-width copy.
                for s in range(2):
                    c0, c1 = s * QW, (s + 1) * QW
                    nc.vector.tensor_scalar_add(
                        kt_all[:, 3 * ST + c0:3 * ST + c1],
                        pkv[0:H, c0:c1], bvk[0:H, :])
                nc.vector.tensor_scalar_add(st_["vt"][64:128, :],
                                            pkv[64:128, :],
                                            bvk[64:128, :])
            return [f]

        def emit_T(t, s):
            def f():
                vt = proj_states[t]["vt"]
                pv = psC.tile([128, 2, H], BF16, tag="small",
                              name=f"pvt_{t}_{s}")
                for i in range(2):
                    sb = 2 * s + i
                    nc.tensor.transpose(pv[:, i, :],
                                        vt[64:128,
                                           sb * 128:(sb + 1) * 128],
                                        identb[64:128, :])
                blk = t * 4 + 2 * s
                nc.vector.tensor_copy(v_all[:, blk:blk + 2, 0:H], pv)
            return [f]

        def emit_out(t, ppv):
            """Output closures for round t: copy pv^T out; host divides."""
            state = {}

            def copy():
                state["pv_sb"] = opool.tile([H + 1, QW], F32, tag="pv_sb",
                                            name=f"pvsb_{t}")
                nc.vector.tensor_copy(state["pv_sb"], ppv)

            def dma():
                nc.sync.dma_start(out=y_d[:, t, :], in_=state["pv_sb"])

            return [copy, dma]

        # ---- static filler schedule (slot -> closure groups) ----
        # Placement tracks the xt DMA arrival order:
        # x0,[w],x1,x2,[mask01],x3,[mask23],x4,x6,x5,x7
        # T(t,s) sits >=1 slot after its K-half so the transpose LDW
        # never waits on the DVE vt copy.
        schedule = {
            0: emit_T(0, 0),
            1: emit_A(1) + emit_T(0, 1),
            2: emit_K(1, 0),
            4: emit_K(1, 1),
            5: emit_A(2) + emit_T(1, 0),
            6: emit_K(2, 0) + emit_T(1, 1),
            9: emit_K(2, 1),
            10: emit_T(2, 0),
            11: emit_A(3) + emit_T(2, 1),
            14: emit_K3(),
            16: emit_T(3, 0),
            17: emit_T(3, 1),
        }

        # prologue: Q pass + both KV halves of round 0 (K(0,1) covers
        # the DVE kt-add latency before the first scores pair)
        for f in emit_A(0) + emit_K(0, 0) + emit_K(0, 1):
            f()

        outq = deque()
        plist = {}
        ppvs = {}

        for g in range(len(PAIR_ORDER) + PDEPTH):
            if g < len(PAIR_ORDER):
                t, u = PAIR_ORDER[g]
                # scores pair: 2 k-blocks into one [128, 512] PSUM tile
                ps = psA.tile([128, PW], F32, tag="big", name=f"ps_{g}")
                for half in range(2):
                    kb = 2 * u + half
                    nc.tensor.matmul(
                        ps[:, half * QW:(half + 1) * QW],
                        kt_all[:, kb * 128:(kb + 1) * 128],
                        qt_all[:, t, :], start=True, stop=True)
                p_sb = ppool.tile([128, PW], BF16, tag="p", name=f"p_{g}")
                if u == 2 * t + 1:
                    # off-diagonal pair: mask is all-ones (h=1) or
                    # all-zeros (h=0) — fold into the exp bias.
                    nc.scalar.activation(p_sb, ps, AF.Exp, scale=0.125,
                                         bias=selb)
                else:
                    nc.scalar.activation(p_sb, ps, AF.Exp, scale=0.125)
                    if u == 2 * t:
                        # diagonal pair: causal mask is core-uniform:
                        # keep iff f - 128*j' - p >= 0 (p = key row,
                        # j' = k-block half, f = query col).
                        nc.gpsimd.affine_select(
                            p_sb, p_sb, [[-128, 2], [1, QW]],
                            mybir.AluOpType.is_ge, 0.0, base=0,
                            channel_multiplier=-1)
                plist[g] = p_sb
                for f in schedule.get(g, ()):  # chunk-gated filler
                    f()
                for _ in range(min(2, len(outq))):
                    outq.popleft()()
            j = g - PDEPTH
            if j >= 0:
                tj, uj = PAIR_ORDER[j]
                if uj == 0:
                    ppvs[tj] = psB.tile([H + 1, QW], F32, tag="p65",
                                        name=f"ppv_{tj}")
                last_u = LAST_PAIR[tj]
                for half in range(2):
                    kb = 2 * uj + half
                    nc.tensor.matmul(ppvs[tj], v_all[:, kb, :],
                                     plist[j][:, half * QW:(half + 1) * QW],
                                     start=(kb == 0),
                                     stop=(uj == last_u and half == 1))
                del plist[j]
                if uj == last_u:
                    outq.extend(emit_out(tj, ppvs[tj]))
        while outq:
            outq.popleft()()

    nc.compile()
    return nc


_NC_CACHE = None


def _get_nc():
    global _NC_CACHE
    if _NC_CACHE is None:
        _NC_CACHE = build_program()
    return _NC_CACHE


def make_host_inputs(x, Wq, bq, Wk, bk, Wv, bv):
    """Per-core input maps from the full problem inputs."""
    x = np.asarray(x, np.float32)
    wkv = np.hstack([np.asarray(Wk, np.float32), np.asarray(Wv, np.float32)])
    wkv_t = wkv.astype(NPBF16).reshape(NEC, 128, 128).transpose(1, 0, 2)
    wq_t = (np.asarray(Wq, np.float32).astype(NPBF16)
            .reshape(NEC, 128, H).transpose(1, 0, 2))

    # mask[p, j, f] = 1 iff query(256h + f) >= key(koff_h[j] + p), offsets
    # within the 512-tile in ORIGINAL order; device k-block 4t+j holds
    # original offset koff_h[j] after the per-core permutation.
    wrest = np.ascontiguousarray(wkv_t.reshape(128, NEC * 128))
    heads = []
    for h in range(2):
        bcols = np.zeros((128, 3), NPBF16)
        bcols[0:H, 0] = np.asarray(bk, np.float32).astype(NPBF16)
        bcols[H:128, 0] = np.asarray(bv, np.float32).astype(NPBF16)
        bcols[0:H, 1] = np.asarray(bq, np.float32).astype(NPBF16)
        bcols[:, 2] = 0.0 if h == 1 else -10000.0
        heads.append(np.concatenate(
            [wq_t.reshape(128, NEC * H), bcols], axis=1))

    # x^T per (batch, half): device s-block g holds original block perm[g]
    maps = []
    for c in range(NCORES):
        b, h = c // 2, c % 2
        xtb = np.ascontiguousarray(x[b].astype(NPBF16).T)    # [E, S]
        if h == 1:
            blocks = xtb.reshape(E, S // 128, 128)
            # within each 512-tile: device [0,1,2,3] = orig [2,3,0,1]
            perm = np.arange(S // 128).reshape(-1, 4)[:, [2, 3, 0, 1]].ravel()
            xtb = np.ascontiguousarray(blocks[:, perm, :].reshape(E, S))
        # [128, chunk, ec, col]: each chunk is a contiguous 4 KiB
        # per-partition run (DMA line rate); matmuls read 256-col slabs.
        xt_t = np.ascontiguousarray(
            xtb.reshape(NEC, 128, NCH, XCH).transpose(1, 2, 0, 3))
        # head = [wq | biases | x-chunk0]: one contiguous
        # first DMA unblocking warmup -> Q pass with minimum latency.
        head = np.ascontiguousarray(np.concatenate(
            [heads[h], xt_t[:, 0].reshape(128, NEC * XCH)], axis=1))
        maps.append({"head": head, "wrest": wrest,
                     "xt": np.ascontiguousarray(xt_t[:, 1:])})
    return maps


def assemble_output(results):
    """results: per-core {'y': [65, 4, 256]} keyed 0..7; host divides."""
    out = np.empty((B, S, H), np.float32)
    for c in range(NCORES):
        b, h = c // 2, c % 2
        y = np.asarray(results[c]["y"], np.float32)  # [65, t, f]
        o = (y[0:H] / y[H:H + 1]).transpose(1, 2, 0)  # [t, f, H]
        for t in range(NST):
            out[b, 512 * t + 256 * h: 512 * t + 256 * h + 256, :] = o[t]
    return out


def run_cores(in_maps, trace=False):
    from concourse.bass_utils import run_bass_kernel_spmd
    nc = _get_nc()
    return run_bass_kernel_spmd(nc, in_maps, list(range(NCORES)), trace=trace)


def kernel(x, Wq, bq, Wk, bk, Wv, bv):
    in_maps = make_host_inputs(x, Wq, bq, Wk, bk, Wv, bv)
    res = run_cores(in_maps).results
    return assemble_output(res)



# revision 56
# speedup vs baseline: 1.1813x; 1.1813x over previous
"""Trainium2 Bass kernel for single-head causal attention.

Problem: B=4, S=2048, E=1024, H=64 fp32.
  q = x@Wq+bq; k = x@Wk+bk; v = x@Wv+bv
  out = softmax(causal(q k^T / sqrt(H))) v

Sharding: 8 cores; core c = (batch b=c//2, query-half h=c%2).
Each core computes full K/V for its batch but attention for only its
1024 queries (h=0: front 256 of each 512-tile, h=1: back 256).
SPMD-uniform: the per-core query selection is realized by a host-side
128-column block permutation of x^T (h=1 swaps the half-tiles within
each 512 tile), and causality by per-core mask tables; the device
program is identical on all cores.

All matmul operands are bf16 (1 cycle/col on the PE vs 4 for fp32),
fp32 accumulation in PSUM. x is transposed and cast to bf16 on the
host: no PE transposes of x, and DMA traffic halves (4MB/core).
The device returns pv^T tiles [65, 256] per q-tile (row 64 = softmax
denominator from a ones-column in V); the host does the final
divide + transpose, removing the whole output transpose stage.

Engine assignment: PE = projections + scores + PV + V transposes
(plus a HAM-warmup matmul burst during the initial DMA wait);
ACT = exp (512-wide pairs) + Q copy; DVE = K/V copies, masks, V-block
copies, pv copies, bias cast. K^T/Q^T live on partitions 64:127
([Wv|Wk] packing) so every PSUM->SBUF copy is partition-aligned.
(gpsimd cannot touch PSUM on HW, so it only does memsets.)

The kernel is one software-pipelined stream over 20 score/PV "pairs"
(2 k-blocks each); rounds 2 and 3 are interleaved pair-by-pair so the
exp (ACT) load of round 3 overlaps PE-heavy projection work, and an
unmasked pair closes round 3 so the final exp->mask->PV tail is short.
Projection closures for future rounds (split into chunk-gated groups
A/K1/K2/T placed at slots matching the xt DMA arrival order) and
output stages are used as PE filler between pairs, so the in-order PE
queue never stalls on the exp->mask->PV chain (PDEPTH=2 lookahead).
PSUM banks (8): scores 2 + pkv 1 + ppv 2 + pq 1 + V-trans 2.
"""

import sys
from collections import deque
from contextlib import ExitStack

import numpy as np
import ml_dtypes

if "/opt/trn_rl_repo" not in sys.path:
    sys.path.insert(0, "/opt/trn_rl_repo")

import concourse.bacc as bacc
import concourse.mybir as mybir
import concourse.tile as tile

B, S, E, H = 4, 2048, 1024, 64
NCORES = 8
F32 = mybir.dt.float32
BF16 = mybir.dt.bfloat16
AF = mybir.ActivationFunctionType
NPBF16 = ml_dtypes.bfloat16

ST = 512          # s-tile width (projections / one round)
NST = S // ST     # 4 rounds
NEC = E // 128    # 8 e-chunks (contraction)
QW = 256          # per-core q-tile width in attention
PW = 2 * QW       # paired width (2 k-blocks per exp)
XCH = 256         # xt DMA chunk width
NCH = S // XCH    # 8 chunks
PDEPTH = 3        # attention software-pipeline depth, in pairs
NWARM = 8         # HAM-warmup matmuls (512 cols each)

# head tensor columns: [wq | biases+sel | x-chunk0] — one contiguous
# first DMA so the Q pass unblocks as early as possible. (identb is
# generated on device.)
H_Q = 0
H_BVK = H_Q + NEC * H
H_X0 = H_BVK + 3
HEADC = H_X0 + NEC * XCH
# wrest tensor: [wkv] (causal masks are generated on device:
# affine_select for the diagonal pairs, exp-bias for the off pairs)
R_KV = 0
RESTC = R_KV + NEC * 128

# rounds 2/3 interleaved (round-3 exp load overlaps round-2+proj PE work),
# round 3 starting after its Q-pass chunk (x6); the final pairs are
# ordered so an unmasked pair closes round 3 (shorter tail chain).
PAIR_ORDER = [(0, 0), (0, 1),
              (1, 0), (1, 1), (1, 2), (1, 3),
              (2, 0), (2, 1), (2, 2), (3, 0), (2, 3), (3, 1), (2, 4),
              (3, 2), (2, 5), (3, 3), (3, 6), (3, 7), (3, 5), (3, 4)]
# last-emitted pair per round (closes that round's PV accumulation)
LAST_PAIR = {0: 1, 1: 3, 2: 5, 3: 4}


def build_program():
    nc = bacc.Bacc("TRN2", target_bir_lowering=False, debug=False,
                   num_devices=NCORES)

    hd_d = nc.dram_tensor("head", [128, HEADC], BF16, kind="ExternalInput")
    wr_d = nc.dram_tensor("wrest", [128, RESTC], BF16, kind="ExternalInput")
    xt_d = nc.dram_tensor("xt", [128, NCH - 1, NEC, XCH], BF16,
                          kind="ExternalInput")
    y_d = nc.dram_tensor("y", [H + 1, NST, QW], F32, kind="ExternalOutput")

    with tile.TileContext(nc) as tc, ExitStack() as ctx:
        singles = ctx.enter_context(tc.tile_pool(name="singles", bufs=1))
        vtpool = ctx.enter_context(tc.tile_pool(name="vtpool", bufs=2))
        ppool = ctx.enter_context(tc.tile_pool(name="ppool", bufs=5))
        opool = ctx.enter_context(tc.tile_pool(name="opool", bufs=4))
        # PSUM 8 banks: sc-pairs 2 + kv-halves 2 + ppv 2 + pq 1 + vtrans 1
        psA = ctx.enter_context(tc.tile_pool(name="psA", bufs=2, space="PSUM"))
        psKV = ctx.enter_context(tc.tile_pool(name="psKV", bufs=2,
                                              space="PSUM"))
        psB = ctx.enter_context(tc.tile_pool(name="psB", bufs=2, space="PSUM"))
        psQ = ctx.enter_context(tc.tile_pool(name="psQ", bufs=1, space="PSUM"))
        psC = ctx.enter_context(tc.tile_pool(name="psC", bufs=1, space="PSUM"))

        # ---- SBUF tiles ----
        hd = singles.tile([128, HEADC], BF16)
        wr = singles.tile([128, RESTC], BF16)
        xt = singles.tile([128, NCH - 1, NEC, XCH], BF16)

        # ---- DMAs, in dependency-priority order (one sync ring =
        # strict FIFO = bandwidth priority). All runs are contiguous
        # multi-KiB per partition.
        def xt_dma(xc):
            nc.sync.dma_start(out=xt[:, xc - 1], in_=xt_d[:, xc - 1])

        # head in two pieces: the Q pass can start on [wq|x0 ec0-3]
        # while [x0 ec4-7] still streams.
        H1C = H_X0 + (NEC // 2) * XCH
        nc.sync.dma_start(out=hd[:, 0:H1C], in_=hd_d[:, 0:H1C])
        nc.sync.dma_start(out=hd[:, H1C:HEADC], in_=hd_d[:, H1C:HEADC])
        nc.sync.dma_start(out=wr[:, R_KV:RESTC], in_=wr_d[:, R_KV:RESTC])
        # x1 in ec-halves: K(0,1)'s first matmuls unblock half a
        # transfer earlier.
        nc.sync.dma_start(out=xt[:, 0, 0:4], in_=xt_d[:, 0, 0:4])
        nc.sync.dma_start(out=xt[:, 0, 4:8], in_=xt_d[:, 0, 4:8])
        xt_dma(2)
        xt_dma(3)
        xt_dma(4)
        xt_dma(6)   # round-3 Q-pass chunk before the round-2/3 k-tails
        xt_dma(5)
        xt_dma(7)

        def wkv_ap(ec):   # [Wv | Wk] chunk: out rows 0:64 = V, 64:128 = K
            return wr[:, R_KV + ec * 128: R_KV + (ec + 1) * 128]

        def wq_ap(ec):
            return hd[:, H_Q + ec * H: H_Q + (ec + 1) * H]

        def xt_ap(ch, ec):  # [128, 256] moving slab of x^T
            if ch == 0:
                return hd[:, H_X0 + ec * XCH: H_X0 + (ec + 1) * XCH]
            return xt[:, ch - 1, ec, :]

        # identity (for PE transposes) built on device on partitions
        # 64:128 (same base as the vt stationary): ones, then keep only
        # the diagonal (iota p - c == 0; partition index is view-rel).
        identb = singles.tile([128, H], BF16)
        nc.gpsimd.memset(identb[64:128, :], 1.0)
        nc.gpsimd.affine_select(identb[64:128, :], identb[64:128, :],
                                [[-1, H]], mybir.AluOpType.is_equal,
                                0.0, base=0, channel_multiplier=1)
        # HAM warmup: dummy matmuls on a just-memset tile run while the
        # first DMAs stream, so the PE clock-gate is already released
        # (and the sim's p-state ramp elapsed) when real work arrives.
        # 512-col matmuls keep the PE busy window unbroken until the
        # first xt chunk lands (~2.5 us after the PE clears its
        # preamble).
        warm = singles.tile([128, 512], BF16)
        nc.vector.memset(warm, 0.0)
        for w in range(NWARM):
            pw_ = psA.tile([128, PW], F32, tag="big", name=f"warm_{w}")
            nc.tensor.matmul(pw_, warm[:, 0:128], warm,
                             start=True, stop=True)
        # biases ride in the head as bf16; engines need fp32 scalars
        fb = singles.tile([128, 3], F32)
        nc.vector.tensor_copy(fb, hd[:, H_BVK:H_BVK + 3])
        bvk = fb[:, 0:1]        # rows 0:64 = bv, rows 64:128 = bk
        bq = fb[0:H, 1:2]
        # per-core selector bias for off-diagonal pairs: 0 keeps the
        # pair (h=1: all-causal), -10000 zeroes it via exp underflow
        # (h=0: fully masked).
        selb = fb[:, 2:3]

        qt_all = singles.tile([64, NST, QW], BF16)   # own Q^T (parts 0:64)
        kt_all = singles.tile([64, S], BF16)         # K^T (parts 0:64)
        v_all = singles.tile([128, S // 128, H + 1], BF16)  # V blocks
        nc.vector.memset(v_all[:, :, H:H + 1], 1.0)

        # ---- emission helpers ----
        # Projection work for round t is split into chunk-gated groups so
        # the in-order PE queue never blocks on a not-yet-arrived xt chunk:
        #   A(t)  = Q pass + qt copy            (needs xt chunk 2t)
        #   K1(t) = KV pass cols 0:256 + copies (needs xt chunk 2t)
        #   K2(t) = KV pass cols 256:512        (needs xt chunk 2t+1)
        #   T1/T2(t) = V-block transposes for each half
        proj_states = {}

        def emit_A(t):
            def f():
                st_ = proj_states.setdefault(t, {})
                st_["pq"] = psQ.tile([H, QW], F32, tag="pq",
                                     name=f"pq_{t}")
                for ec in range(NEC):
                    nc.tensor.matmul(st_["pq"], wq_ap(ec),
                                     xt_ap(2 * t, ec),
                                     start=(ec == 0), stop=(ec == NEC - 1))
                # partition-aligned DVE copy (0:64 -> 0:64); keeps the
                # qt chain off the ACT queue where exps would delay it.
                nc.vector.tensor_scalar_add(qt_all[:, t, :], st_["pq"], bq)
            return [f]

        def emit_K(t, s):
            c0, c1 = s * QW, (s + 1) * QW
            def f():
                st_ = proj_states.setdefault(t, {})
                if s == 0:
                    st_["vt"] = vtpool.tile([128, ST], BF16, tag="vt",
                                            name=f"vt_{t}")
                # per-half PSUM tile: its bank is free of the other
                # half's DVE reads, so K2 matmuls never serialize
                # against K1's kt/vt copies.
                pkv = psKV.tile([128, QW], F32, tag="kv",
                                name=f"pkv_{t}_{s}")
                for ec in range(NEC):
                    nc.tensor.matmul(pkv, wkv_ap(ec),
                                     xt_ap(2 * t + s, ec),
                                     start=(ec == 0), stop=(ec == NEC - 1))
                # kt first: scores depend on it; vt only feeds the
                # later V-transpose stage. [Wk|Wv] packing: K = rows
                # 0:64 -> kt_all parts 0:64, V = rows 64:128 -> vt
                # parts 64:128; both copies partition-aligned.
                nc.vector.tensor_scalar_add(
                    kt_all[:, t * ST + c0:t * ST + c1],
                    pkv[0:H, :], bvk[0:H, :])
                nc.vector.tensor_scalar_add(st_["vt"][64:128, c0:c1],
                                            pkv[64:128, :],
                                            bvk[64:128, :])
            return [f]

        def emit_K3():
            # round 3's KV pass arrives last (chunks 6,7 both present
            # by then): one 512-col-moving matmul chain halves the
            # instruction count and hides every LDWEIGHTS.
            def f():
                st_ = proj_states.setdefault(3, {})
                st_["vt"] = vtpool.tile([128, ST], BF16, tag="vt",
                                        name="vt_3")
                pkv = psKV.tile([128, ST], F32, tag="kv", name="pkv_3m")
                for ec in range(NEC):
                    nc.tensor.matmul(pkv, wkv_ap(ec), xt[:, 5:7, ec, :],
                                     start=(ec == 0), stop=(ec == NEC - 1))
                # kt halves separately: scores (3,6) needs only blocks
                # 12,13 and shouldn't wait a full-width copy.
                for s in range(2):
                    c0, c1 = s * QW, (s + 1) * QW
                    nc.vector.tensor_scalar_add(
                        kt_all[:, 3 * ST + c0:3 * ST + c1],
                        pkv[0:H, c0:c1], bvk[0:H, :])
                nc.vector.tensor_scalar_add(st_["vt"][64:128, :],
                                            pkv[64:128, :],
                                            bvk[64:128, :])
            return [f]

        def emit_T(t, s):
            def f():
                vt = proj_states[t]["vt"]
                pv = psC.tile([128, 2, H], BF16, tag="small",
                              name=f"pvt_{t}_{s}")
                for i in range(2):
                    sb = 2 * s + i
                    nc.tensor.transpose(pv[:, i, :],
                                        vt[64:128,
                                           sb * 128:(sb + 1) * 128],
                                        identb[64:128, :])
                blk = t * 4 + 2 * s
                nc.vector.tensor_copy(v_all[:, blk:blk + 2, 0:H], pv)
            return [f]

        def emit_out(t, ppv):
            """Output closures for round t: copy pv^T out; host divides."""
            state = {}

            def copy():
                state["pv_sb"] = opool.tile([H + 1, QW], F32, tag="pv_sb",
                                            name=f"pvsb_{t}")
                nc.vector.tensor_copy(state["pv_sb"], ppv)

            def dma():
                nc.sync.dma_start(out=y_d[:, t, :], in_=state["pv_sb"])

            return [copy, dma]

        # ---- static filler schedule (slot -> closure groups) ----
        # Placement tracks the xt DMA arrival order:
        # x0,[w],x1,x2,[mask01],x3,[mask23],x4,x6,x5,x7
        # T(t,s) sits >=1 slot after its K-half so the transpose LDW
        # never waits on the DVE vt copy.
        schedule = {
            0: emit_T(0, 0),
            1: emit_A(1) + emit_T(0, 1),
            2: emit_K(1, 0),
            4: emit_K(1, 1),
            5: emit_A(2) + emit_T(1, 0),
            6: emit_K(2, 0) + emit_T(1, 1),
            7: emit_A(3),
            10: emit_K(2, 1) + emit_T(2, 0),
            12: emit_T(2, 1),
            15: emit_K3(),
            16: emit_T(3, 0),
            17: emit_T(3, 1),
        }

        # prologue: Q pass + both KV halves of round 0 (K(0,1) covers
        # the DVE kt-add latency before the first scores pair)
        for f in emit_A(0) + emit_K(0, 0) + emit_K(0, 1):
            f()

        outq = deque()
        plist = {}
        ppvs = {}

        for g in range(len(PAIR_ORDER) + PDEPTH):
            if g < len(PAIR_ORDER):
                t, u = PAIR_ORDER[g]
                # scores pair: 2 k-blocks into one [128, 512] PSUM tile
                ps = psA.tile([128, PW], F32, tag="big", name=f"ps_{g}")
                for half in range(2):
                    kb = 2 * u + half
                    nc.tensor.matmul(
                        ps[:, half * QW:(half + 1) * QW],
                        kt_all[:, kb * 128:(kb + 1) * 128],
                        qt_all[:, t, :], start=True, stop=True)
                p_sb = ppool.tile([128, PW], BF16, tag="p", name=f"p_{g}")
                if u == 2 * t + 1:
                    # off-diagonal pair: mask is all-ones (h=1) or
                    # all-zeros (h=0) — fold into the exp bias.
                    nc.scalar.activation(p_sb, ps, AF.Exp, scale=0.125,
                                         bias=selb)
                else:
                    nc.scalar.activation(p_sb, ps, AF.Exp, scale=0.125)
                    if u == 2 * t:
                        # diagonal pair: causal mask is core-uniform:
                        # keep iff f - 128*j' - p >= 0 (p = key row,
                        # j' = k-block half, f = query col).
                        nc.gpsimd.affine_select(
                            p_sb, p_sb, [[-128, 2], [1, QW]],
                            mybir.AluOpType.is_ge, 0.0, base=0,
                            channel_multiplier=-1)
                plist[g] = p_sb
                for f in schedule.get(g, ()):  # chunk-gated filler
                    f()
                for _ in range(min(2, len(outq))):
                    outq.popleft()()
            j = g - PDEPTH
            if j >= 0:
                tj, uj = PAIR_ORDER[j]
                if uj == 0:
                    ppvs[tj] = psB.tile([H + 1, QW], F32, tag="p65",
                                        name=f"ppv_{tj}")
                last_u = LAST_PAIR[tj]
                for half in range(2):
                    kb = 2 * uj + half
                    nc.tensor.matmul(ppvs[tj], v_all[:, kb, :],
                                     plist[j][:, half * QW:(half + 1) * QW],
                                     start=(kb == 0),
                                     stop=(uj == last_u and half == 1))
                del plist[j]
                if uj == last_u:
                    outq.extend(emit_out(tj, ppvs[tj]))
        while outq:
            outq.popleft()()

    nc.compile()
    return nc


_NC_CACHE = None


def _get_nc():
    global _NC_CACHE
    if _NC_CACHE is None:
        _NC_CACHE = build_program()
    return _NC_CACHE


def make_host_inputs(x, Wq, bq, Wk, bk, Wv, bv):
    """Per-core input maps from the full problem inputs."""
    x = np.asarray(x, np.float32)
    wkv = np.hstack([np.asarray(Wk, np.float32), np.asarray(Wv, np.float32)])
    wkv_t = wkv.astype(NPBF16).reshape(NEC, 128, 128).transpose(1, 0, 2)
    wq_t = (np.asarray(Wq, np.float32).astype(NPBF16)
            .reshape(NEC, 128, H).transpose(1, 0, 2))

    # mask[p, j, f] = 1 iff query(256h + f) >= key(koff_h[j] + p), offsets
    # within the 512-tile in ORIGINAL order; device k-block 4t+j holds
    # original offset koff_h[j] after the per-core permutation.
    wrest = np.ascontiguousarray(wkv_t.reshape(128, NEC * 128))
    heads = []
    for h in range(2):
        bcols = np.zeros((128, 3), NPBF16)
        bcols[0:H, 0] = np.asarray(bk, np.float32).astype(NPBF16)
        bcols[H:128, 0] = np.asarray(bv, np.float32).astype(NPBF16)
        bcols[0:H, 1] = np.asarray(bq, np.float32).astype(NPBF16)
        bcols[:, 2] = 0.0 if h == 1 else -10000.0
        heads.append(np.concatenate(
            [wq_t.reshape(128, NEC * H), bcols], axis=1))

    # x^T per (batch, half): device s-block g holds original block perm[g]
    maps = []
    for c in range(NCORES):
        b, h = c // 2, c % 2
        xtb = np.ascontiguousarray(x[b].astype(NPBF16).T)    # [E, S]
        if h == 1:
            blocks = xtb.reshape(E, S // 128, 128)
            # within each 512-tile: device [0,1,2,3] = orig [2,3,0,1]
            perm = np.arange(S // 128).reshape(-1, 4)[:, [2, 3, 0, 1]].ravel()
            xtb = np.ascontiguousarray(blocks[:, perm, :].reshape(E, S))
        # [128, chunk, ec, col]: each chunk is a contiguous 4 KiB
        # per-partition run (DMA line rate); matmuls read 256-col slabs.
        xt_t = np.ascontiguousarray(
            xtb.reshape(NEC, 128, NCH, XCH).transpose(1, 2, 0, 3))
        # head = [wq | biases | x-chunk0]: one contiguous
        # first DMA unblocking warmup -> Q pass with minimum latency.
        head = np.ascontiguousarray(np.concatenate(
            [heads[h], xt_t[:, 0].reshape(128, NEC * XCH)], axis=1))
        maps.append({"head": head, "wrest": wrest,
                     "xt": np.ascontiguousarray(xt_t[:, 1:])})
    return maps


def assemble_output(results):
    """results: per-core {'y': [65, 4, 256]} keyed 0..7; host divides."""
    out = np.empty((B, S, H), np.float32)
    for c in range(NCORES):
        b, h = c // 2, c % 2
        y = np.asarray(results[c]["y"], np.float32)  # [65, t, f]
        o = (y[0:H] / y[H:H + 1]).transpose(1, 2, 0)  # [t, f, H]
        for t in range(NST):
            out[b, 512 * t + 256 * h: 512 * t + 256 * h + 256, :] = o[t]
    return out


def run_cores(in_maps, trace=False):
    from concourse.bass_utils import run_bass_kernel_spmd
    nc = _get_nc()
    return run_bass_kernel_spmd(nc, in_maps, list(range(NCORES)), trace=trace)


def kernel(x, Wq, bq, Wk, bk, Wv, bv):
    in_maps = make_host_inputs(x, Wq, bq, Wk, bk, Wv, bv)
    res = run_cores(in_maps).results
    return assemble_output(res)

